# revision 4
# baseline (speedup 1.0000x reference)
"""Cosine-similarity attention kernel for Trainium2 (8 NeuronCores, SPMD).

Problem: query [16,16,1,128], key [16,16,4096,128], mask [16,4096] int32
  scores[b,h,l] = <q,k_l> / max(|q||k_l|, 1e-8);  masked softmax over l.
Output: p_attn [16,16,4096] float32.  Measured: ~98.8us/exec, rel err 8.1e-3.

Sharding: batch dim split across 8 cores (2 batches/core, 32 (b,h) rows).
Host staging (layout/precision only -- dots, norms, rsqrt, exp and the
masked softmax all run on device):
  - K^T re-laid-out j-major as [j, d, bh, 512] bf16 so each j-block is one
    contiguous 4MB stream matching the psum accumulation order (256 small
    DMAs were fixed-cost bound at ~155us; 8 big ones run at line rate),
  - masked stationaries MQ/MONES [d, bh, col] bf16 (q_bh / ones in column
    bh, zeros elsewhere),
  - mask head-replicated to [bh, l] bf16.

Per-core dataflow (l = j*512 + p):
  - per j: one 4MB KT load split across BOTH HWDGE rings (sync + scalar)
  - k2t = kt*kt elementwise per bh (DVE 2x bf16 / ACT Square, 24:8 split)
  - dots:  accumulate 32 masked-Q matmuls  (stationary mq[:,bh,:]) -> psum [32,512]
  - norms: accumulate 32 masked-1s matmuls (stationary mones[:,bh,:]) -> psum [32,512]
    Both land directly in [bh, l] layout; PE does no transposes at all.
  - per-j epilogue: ACT drains psn via Ln(psn*qn2) (qn2 fused into scale),
    rk = exp(-0.5*ln); DVE drains psd fused with the rk product into bf16
    scores; ACT exp; one DVE op does e*mask + per-row partial sums.
  - tail: p = e / sum(e) in 4 chunks alternating DVE/ACT, each chunk's
    store overlapping the next chunk's normalize; output stored bf16 and
    upcast to f32 on the host.

softmax max-subtraction is dropped: scores are cosine similarities in [-1,1],
masked entries are multiplied by 0 after exp (identical to exp(-1e9) -> 0).
"""

import sys

if "/opt/trn_rl_repo" not in sys.path:
    sys.path.insert(0, "/opt/trn_rl_repo")

import numpy as np
import ml_dtypes

import concourse.bacc as bacc
import concourse.tile as tile
from concourse import mybir
from concourse.bass_utils import run_bass_kernel_spmd

F32 = mybir.dt.float32
BF16 = mybir.dt.bfloat16
I32 = mybir.dt.int32
AF = mybir.ActivationFunctionType
AX = mybir.AxisListType

B, H, L, D = 16, 16, 4096, 128
NCORES = 8
BLOC = B // NCORES  # batches per core
NBH = BLOC * H  # 32 (b,h) rows per core
LB = 512  # lambda block size

_ONE_SET = "natural_log_exp_and_others"  # contains Copy/Square/Ln/Exp


class _Bacc(bacc.Bacc):
    """Bacc that pins all activations to a single ACT table set, avoiding
    ~2.7us table reloads when Square and Ln/Exp interleave."""

    PIN_TABLES = True

    def insert_act_table_loads(self):
        super().insert_act_table_loads()
        if not self.PIN_TABLES:
            return
        from concourse.hw_specs import get_activation_tables

        names = list(get_activation_tables(self.m.arch).keys())
        target = names.index(_ONE_SET)
        first = True
        for fn in self.m.functions:
            for blk in fn.blocks:
                keep = []
                changed = False
                for inst in blk.instructions:
                    if type(inst).__name__ == "InstLoadActFuncSet":
                        if first:
                            inst.act_func_set_id = target
                            first = False
                            keep.append(inst)
                        else:
                            changed = True
                        continue
                    keep.append(inst)
                if changed:
                    del blk.instructions[:]
                    for i in keep:
                        blk.instructions.append(i)


def build_module(nj=L // LB, variant="full", reps=1):
    lt = nj * LB  # total l covered (full run: 4096)
    nc = _Bacc(
        "TRN2", target_bir_lowering=False, debug=False, num_devices=NCORES
    )
    q_d = nc.dram_tensor("query", [BLOC, H, 1, D], F32, kind="ExternalInput").ap()
    kt_d = nc.dram_tensor("keyT", [nj, D, NBH, LB], BF16, kind="ExternalInput").ap()
    mq_d = nc.dram_tensor("mq", [D, NBH, NBH], BF16, kind="ExternalInput").ap()
    mo_d = nc.dram_tensor("mones", [D, NBH, NBH], BF16, kind="ExternalInput").ap()
    mf_d = nc.dram_tensor("maskf", [NBH, lt], BF16, kind="ExternalInput").ap()
    o_d = nc.dram_tensor("out", [BLOC, H, lt], BF16, kind="ExternalOutput").ap()

    with tile.TileContext(nc) as tc:
        with (
            tc.tile_pool(name="persist", bufs=1) as pers,
            tc.tile_pool(name="ktp", bufs=3) as ktp,
            tc.tile_pool(name="k2tp", bufs=8) as k2tp,
            tc.tile_pool(name="knp", bufs=2) as knp,
            tc.tile_pool(name="psd", bufs=2, space="PSUM") as psd,
            tc.tile_pool(name="psn", bufs=2, space="PSUM") as psn,
        ):
            # ---------------- prologue: staged constants -----------------
            qsb = pers.tile([NBH, D], F32, tag="qsb")
            nc.sync.dma_start(qsb[:], q_d.rearrange("b h o d -> (b h) (o d)"))

            mq = pers.tile([128, NBH, NBH], BF16, tag="mq")
            nc.sync.dma_start(mq[:], mq_d)
            mones = pers.tile([128, NBH, NBH], BF16, tag="mones")
            nc.sync.dma_start(mones[:], mo_d)
            maskf = pers.tile([NBH, lt], BF16, tag="maskf")
            nc.sync.dma_start(maskf[:], mf_d)

            # qn2[bh] = |q_bh|^2  (fused square+reduce on DVE)
            junkq = pers.tile([NBH, D], F32, tag="junkq")
            qn2 = pers.tile([NBH, 1], F32, tag="qn2")
            nc.vector.scalar_tensor_tensor(
                out=junkq[:],
                in0=qsb[:],
                scalar=1.0,
                in1=qsb[:],
                op0=mybir.AluOpType.mult,
                op1=mybir.AluOpType.mult,
                accum_out=qn2[:],
            )

            scores = pers.tile([NBH, lt], BF16, tag="scores")
            partials = pers.tile([NBH, nj], F32, tag="partials")

            # ---------------- main loop -----------------
            def one_pass():
              for j in range(nj):
                  if variant != "dmaonly":
                      psd_t = psd.tile([NBH, LB], F32, tag="psd")
                      psn_t = psn.tile([NBH, LB], F32, tag="psn")
                  ktj = ktp.tile([128, NBH, LB], BF16, tag="ktj")
                  nc.sync.dma_start(
                      ktj[:, 0 : NBH // 2, :], kt_d[j, :, 0 : NBH // 2, :]
                  )
                  nc.scalar.dma_start(
                      ktj[:, NBH // 2 :, :], kt_d[j, :, NBH // 2 :, :]
                  )
                  for bh in range(NBH if variant != "dmaonly" else 0):
                      kt = ktj[:, bh, :]
                      if variant != "nosq":
                          k2t = k2tp.tile([128, LB], BF16, tag="k2t")
                          # split squares DVE/ACT to balance engine load
                          if bh % 4 == 3:
                              nc.scalar.activation(k2t[:], kt, AF.Square)
                          else:
                              nc.vector.tensor_mul(k2t[:], kt, kt)
                      if variant == "nomm":
                          continue
                      nc.tensor.matmul(
                          psd_t[:],
                          mq[:, bh, :],
                          kt,
                          start=(bh == 0),
                          stop=(bh == NBH - 1),
                      )
                      nc.tensor.matmul(
                          psn_t[:],
                          mones[:, bh, :],
                          kt if variant == "nosq" else k2t[:],
                          start=(bh == 0),
                          stop=(bh == NBH - 1),
                      )

                  sl = slice(j * LB, (j + 1) * LB)
                  kn2d = knp.tile([NBH, LB], F32, tag="kn2d")
                  if variant in ("dmaonly", "nomm"):
                      nc.vector.memset(scores[:, sl], 0.0)
                      nc.vector.scalar_tensor_tensor(
                          out=scores[:, sl],
                          in0=scores[:, sl],
                          scalar=1.0,
                          in1=maskf[:, sl],
                          op0=mybir.AluOpType.mult,
                          op1=mybir.AluOpType.mult,
                          accum_out=partials[:, j : j + 1],
                      )
                      continue

                  # per-j epilogue ([32, 512] ops, overlapped with next j).
                  # ACT drains psn with the qn2 product fused into Ln's scale:
                  # kn2d = ln(psn * qn2); rk = exp(-0.5 * kn2d).
                  nc.scalar.activation(kn2d[:], psn_t[:], AF.Ln, scale=qn2[:])
                  nc.scalar.activation(kn2d[:], kn2d[:], AF.Exp, scale=-0.5)
                  # DVE drains psd fused with the rk product.
                  nc.vector.tensor_mul(scores[:, sl], psd_t[:], kn2d[:])
                  nc.scalar.activation(scores[:, sl], scores[:, sl], AF.Exp)
                  # fused e*mask with per-row partial sums (one DVE op)
                  nc.vector.scalar_tensor_tensor(
                      out=scores[:, sl],
                      in0=scores[:, sl],
                      scalar=1.0,
                      in1=maskf[:, sl],
                      op0=mybir.AluOpType.mult,
                      op1=mybir.AluOpType.mult,
                      accum_out=partials[:, j : j + 1],
                  )

              # ---------------- tail -----------------
              tot = pers.tile([NBH, 1], F32, tag="tot")
              nc.vector.reduce_sum(tot[:], partials[:], axis=AX.X)
              srec = pers.tile([NBH, 1], F32, tag="srec")
              nc.vector.reciprocal(srec[:], tot[:])
              # normalize + store in 4 chunks, alternating DVE/ACT, so each
              # chunk's store overlaps the next chunk's multiply.
              oflat = o_d.rearrange("b h l -> (b h) l")
              CH = lt // 4 if lt >= 4 else lt
              nch = lt // CH
              for t in range(nch):
                  cs = slice(t * CH, (t + 1) * CH)
                  if t % 2 == 0:
                      nc.vector.tensor_scalar_mul(
                          scores[:, cs], scores[:, cs], srec[:]
                      )
                  else:
                      nc.scalar.activation(
                          scores[:, cs], scores[:, cs], AF.Copy, scale=srec[:]
                      )
                  nc.sync.dma_start(oflat[:, cs], scores[:, cs])

            if reps == 1:
                one_pass()
            else:
                with tc.For_i(0, reps, 1):
                    one_pass()

    nc.compile()
    return nc


_CACHE = {}


def _get_module(nj=L // LB, variant="full"):
    key = (nj, variant)
    if key not in _CACHE:
        _CACHE[key] = build_module(nj, variant)
    return _CACHE[key]


def stage_inputs(query, key, mask, nj=L // LB):
    """Host-side staging: shard over cores; K^T bf16, masked stationaries,
    head-replicated mask (layout/precision only)."""
    lt = nj * LB
    query = np.asarray(query)
    key = np.asarray(key)
    mask = np.asarray(mask)
    bh_idx = np.arange(NBH)
    mones = np.zeros((D, NBH, NBH), ml_dtypes.bfloat16)
    mones[:, bh_idx, bh_idx] = 1.0
    in_maps = []
    for c in range(NCORES):
        b0 = c * BLOC
        ks = key[b0 : b0 + BLOC, :, :lt, :].astype(ml_dtypes.bfloat16)
        # [bh, j, l', d] -> [j, d, bh, l']
        kj = ks.reshape(NBH, lt // LB, LB, D)
        kt = np.ascontiguousarray(kj.transpose(1, 3, 0, 2))
        qc = query[b0 : b0 + BLOC].reshape(NBH, D)  # [bh, d] f32
        mq = np.zeros((D, NBH, NBH), ml_dtypes.bfloat16)
        mq[:, bh_idx, bh_idx] = qc.T.astype(ml_dtypes.bfloat16)
        mf = np.repeat(
            mask[b0 : b0 + BLOC, :lt].astype(ml_dtypes.bfloat16), H, axis=0
        )  # [bh, l]
        in_maps.append(
            {
                "query": np.ascontiguousarray(query[b0 : b0 + BLOC], np.float32),
                "keyT": kt,
                "mq": mq,
                "mones": mones,
                "maskf": mf,
            }
        )
    return in_maps


def _run(query, key, mask, trace=False, nj=L // LB):
    nc = _get_module(nj)
    in_maps = stage_inputs(query, key, mask, nj)
    res = run_bass_kernel_spmd(
        nc, in_maps, core_ids=list(range(NCORES)), trace=trace
    )
    out = np.concatenate([r["out"] for r in res.results], axis=0).astype(np.float32)
    return out, res


def kernel(query, key, mask):
    out, _ = _run(np.asarray(query), np.asarray(key), np.asarray(mask))
    return out


# revision 5
# speedup vs baseline: 1.5438x; 1.5438x over previous
"""Cosine-similarity attention kernel for Trainium2 (8 NeuronCores, SPMD).

Problem: query [16,16,1,128], key [16,16,4096,128], mask [16,4096] int32
  scores[b,h,l] = <q,k_l> / max(|q||k_l|, 1e-8);  masked softmax over l.
Output: p_attn [16,16,4096] float32.  Measured: ~109-137us/exec (device
power-state dependent), rel err 8.1e-3.

Sharding: batch dim split across 8 cores (2 batches/core, 32 (b,h) rows).
Host staging (layout/precision only -- dots, norms, rsqrt, exp and the
masked softmax all run on device):
  - K^T re-laid-out j-major as [j, d, bh, 512] bf16 so each j-block is one
    contiguous 4MB stream matching the psum accumulation order (256 small
    DMAs were fixed-cost bound at ~155us; 8 big ones run at line rate),
  - masked stationaries MQ/MONES [d, bh, col] bf16 (q_bh / ones in column
    bh, zeros elsewhere),
  - mask head-replicated to [bh, l] bf16.

Per-core dataflow (l = j*512 + p):
  - per j: one 4MB KT load split across BOTH HWDGE rings (sync + scalar)
  - k2t = kt*kt elementwise per bh (DVE 2x bf16 / ACT Square, 24:8 split)
  - phase 1: all 32 dots matmuls (stationary mq[:,bh,:]) -> psum [32,512];
    PE order is gated only by the DMA while DVE/ACT produce the squares
  - phase 2: all 32 norms matmuls (stationary mones[:,bh,:]) -> psum [32,512]
    Both land directly in [bh, l] layout; PE does no transposes at all.
  - per-j epilogue: ACT drains psn via Ln(psn*qn2) (qn2 fused into scale),
    rk = exp(-0.5*ln); DVE drains psd fused with the rk product into bf16
    scores; ACT exp; one DVE op does e*mask + per-row partial sums.
  - tail: p = e / sum(e) in 4 chunks alternating DVE/ACT, each chunk's
    store overlapping the next chunk's normalize; output stored bf16 and
    upcast to f32 on the host.

softmax max-subtraction is dropped: scores are cosine similarities in [-1,1],
masked entries are multiplied by 0 after exp (identical to exp(-1e9) -> 0).
"""

import sys

if "/opt/trn_rl_repo" not in sys.path:
    sys.path.insert(0, "/opt/trn_rl_repo")

import numpy as np
import ml_dtypes

import concourse.bacc as bacc
import concourse.tile as tile
from concourse import mybir
from concourse.bass_utils import run_bass_kernel_spmd

F32 = mybir.dt.float32
BF16 = mybir.dt.bfloat16
I32 = mybir.dt.int32
AF = mybir.ActivationFunctionType
AX = mybir.AxisListType

B, H, L, D = 16, 16, 4096, 128
NCORES = 8
BLOC = B // NCORES  # batches per core
NBH = BLOC * H  # 32 (b,h) rows per core
LB = 512  # lambda block size

_ONE_SET = "natural_log_exp_and_others"  # contains Copy/Square/Ln/Exp


class _Bacc(bacc.Bacc):
    """Bacc that pins all activations to a single ACT table set, avoiding
    ~2.7us table reloads when Square and Ln/Exp interleave."""

    PIN_TABLES = True

    def insert_act_table_loads(self):
        super().insert_act_table_loads()
        if not self.PIN_TABLES:
            return
        from concourse.hw_specs import get_activation_tables

        names = list(get_activation_tables(self.m.arch).keys())
        target = names.index(_ONE_SET)
        first = True
        for fn in self.m.functions:
            for blk in fn.blocks:
                keep = []
                changed = False
                for inst in blk.instructions:
                    if type(inst).__name__ == "InstLoadActFuncSet":
                        if first:
                            inst.act_func_set_id = target
                            first = False
                            keep.append(inst)
                        else:
                            changed = True
                        continue
                    keep.append(inst)
                if changed:
                    del blk.instructions[:]
                    for i in keep:
                        blk.instructions.append(i)


def build_module(nj=L // LB, variant="full", reps=1):
    lt = nj * LB  # total l covered (full run: 4096)
    nc = _Bacc(
        "TRN2", target_bir_lowering=False, debug=False, num_devices=NCORES
    )
    q_d = nc.dram_tensor("query", [BLOC, H, 1, D], F32, kind="ExternalInput").ap()
    kt_d = nc.dram_tensor("keyT", [nj, D, NBH, LB], BF16, kind="ExternalInput").ap()
    mq_d = nc.dram_tensor("mq", [D, NBH, NBH], BF16, kind="ExternalInput").ap()
    mo_d = nc.dram_tensor("mones", [D, NBH, NBH], BF16, kind="ExternalInput").ap()
    mf_d = nc.dram_tensor("maskf", [NBH, lt], BF16, kind="ExternalInput").ap()
    o_d = nc.dram_tensor("out", [BLOC, H, lt], BF16, kind="ExternalOutput").ap()

    with tile.TileContext(nc) as tc:
        with (
            tc.tile_pool(name="persist", bufs=1) as pers,
            tc.tile_pool(name="ktp", bufs=2) as ktp,
            tc.tile_pool(name="k2tp", bufs=48) as k2tp,
            tc.tile_pool(name="knp", bufs=2) as knp,
            tc.tile_pool(name="psd", bufs=2, space="PSUM") as psd,
            tc.tile_pool(name="psn", bufs=2, space="PSUM") as psn,
        ):
            # ---------------- prologue: staged constants -----------------
            qsb = pers.tile([NBH, D], F32, tag="qsb")
            nc.sync.dma_start(qsb[:], q_d.rearrange("b h o d -> (b h) (o d)"))

            mq = pers.tile([128, NBH, NBH], BF16, tag="mq")
            nc.sync.dma_start(mq[:], mq_d)
            mones = pers.tile([128, NBH, NBH], BF16, tag="mones")
            nc.sync.dma_start(mones[:], mo_d)
            maskf = pers.tile([NBH, lt], BF16, tag="maskf")
            nc.sync.dma_start(maskf[:], mf_d)

            # qn2[bh] = |q_bh|^2  (fused square+reduce on DVE)
            junkq = pers.tile([NBH, D], F32, tag="junkq")
            qn2 = pers.tile([NBH, 1], F32, tag="qn2")
            nc.vector.scalar_tensor_tensor(
                out=junkq[:],
                in0=qsb[:],
                scalar=1.0,
                in1=qsb[:],
                op0=mybir.AluOpType.mult,
                op1=mybir.AluOpType.mult,
                accum_out=qn2[:],
            )

            scores = pers.tile([NBH, lt], BF16, tag="scores")
            partials = pers.tile([NBH, nj], F32, tag="partials")

            # ---------------- main loop -----------------
            def one_pass():
              for j in range(nj):
                  if variant != "dmaonly":
                      psd_t = psd.tile([NBH, LB], F32, tag="psd")
                      psn_t = psn.tile([NBH, LB], F32, tag="psn")
                  ktj = ktp.tile([128, NBH, LB], BF16, tag="ktj")
                  nc.sync.dma_start(
                      ktj[:, 0 : NBH // 2, :], kt_d[j, :, 0 : NBH // 2, :]
                  )
                  nc.scalar.dma_start(
                      ktj[:, NBH // 2 :, :], kt_d[j, :, NBH // 2 :, :]
                  )
                  # Phase 1: squares (DVE/ACT) + all 32 dots matmuls.
                  # PE program order = execution order, so keeping the
                  # norms matmuls (which wait on k2t) out of the dots
                  # stream lets PE run dots gated only by the DMA while
                  # DVE/ACT produce the squares concurrently.
                  k2ts = []
                  for bh in range(NBH if variant != "dmaonly" else 0):
                      kt = ktj[:, bh, :]
                      if variant != "nosq":
                          k2t = k2tp.tile([128, LB], BF16, tag="k2t")
                          # split squares DVE/ACT to balance engine load
                          if bh % 4 == 3:
                              nc.scalar.activation(k2t[:], kt, AF.Square)
                          else:
                              nc.vector.tensor_mul(k2t[:], kt, kt)
                          k2ts.append(k2t)
                      if variant == "nomm":
                          continue
                      nc.tensor.matmul(
                          psd_t[:],
                          mq[:, bh, :],
                          kt,
                          start=(bh == 0),
                          stop=(bh == NBH - 1),
                      )
                  # Phase 2: all 32 norms matmuls.
                  for bh in range(
                      NBH if variant not in ("dmaonly", "nomm") else 0
                  ):
                      nc.tensor.matmul(
                          psn_t[:],
                          mones[:, bh, :],
                          ktj[:, bh, :] if variant == "nosq" else k2ts[bh][:],
                          start=(bh == 0),
                          stop=(bh == NBH - 1),
                      )

                  sl = slice(j * LB, (j + 1) * LB)
                  kn2d = knp.tile([NBH, LB], F32, tag="kn2d")
                  if variant in ("dmaonly", "nomm"):
                      nc.vector.memset(scores[:, sl], 0.0)
                      nc.vector.scalar_tensor_tensor(
                          out=scores[:, sl],
                          in0=scores[:, sl],
                          scalar=1.0,
                          in1=maskf[:, sl],
                          op0=mybir.AluOpType.mult,
                          op1=mybir.AluOpType.mult,
                          accum_out=partials[:, j : j + 1],
                      )
                      continue

                  # per-j epilogue ([32, 512] ops, overlapped with next j).
                  # ACT drains psn with the qn2 product fused into Ln's scale:
                  # kn2d = ln(psn * qn2); rk = exp(-0.5 * kn2d).
                  nc.scalar.activation(kn2d[:], psn_t[:], AF.Ln, scale=qn2[:])
                  nc.scalar.activation(kn2d[:], kn2d[:], AF.Exp, scale=-0.5)
                  # DVE drains psd fused with the rk product.
                  nc.vector.tensor_mul(scores[:, sl], psd_t[:], kn2d[:])
                  nc.scalar.activation(scores[:, sl], scores[:, sl], AF.Exp)
                  # fused e*mask with per-row partial sums (one DVE op)
                  nc.vector.scalar_tensor_tensor(
                      out=scores[:, sl],
                      in0=scores[:, sl],
                      scalar=1.0,
                      in1=maskf[:, sl],
                      op0=mybir.AluOpType.mult,
                      op1=mybir.AluOpType.mult,
                      accum_out=partials[:, j : j + 1],
                  )

              # ---------------- tail -----------------
              tot = pers.tile([NBH, 1], F32, tag="tot")
              nc.vector.reduce_sum(tot[:], partials[:], axis=AX.X)
              srec = pers.tile([NBH, 1], F32, tag="srec")
              nc.vector.reciprocal(srec[:], tot[:])
              # normalize + store in 4 chunks, alternating DVE/ACT, so each
              # chunk's store overlaps the next chunk's multiply.
              oflat = o_d.rearrange("b h l -> (b h) l")
              CH = lt // 4 if lt >= 4 else lt
              nch = lt // CH
              for t in range(nch):
                  cs = slice(t * CH, (t + 1) * CH)
                  if t % 2 == 0:
                      nc.vector.tensor_scalar_mul(
                          scores[:, cs], scores[:, cs], srec[:]
                      )
                  else:
                      nc.scalar.activation(
                          scores[:, cs], scores[:, cs], AF.Copy, scale=srec[:]
                      )
                  nc.sync.dma_start(oflat[:, cs], scores[:, cs])

            if reps == 1:
                one_pass()
            else:
                with tc.For_i(0, reps, 1):
                    one_pass()

    nc.compile()
    return nc


_CACHE = {}


def _get_module(nj=L // LB, variant="full"):
    key = (nj, variant)
    if key not in _CACHE:
        _CACHE[key] = build_module(nj, variant)
    return _CACHE[key]


def stage_inputs(query, key, mask, nj=L // LB):
    """Host-side staging: shard over cores; K^T bf16, masked stationaries,
    head-replicated mask (layout/precision only)."""
    lt = nj * LB
    query = np.asarray(query)
    key = np.asarray(key)
    mask = np.asarray(mask)
    bh_idx = np.arange(NBH)
    mones = np.zeros((D, NBH, NBH), ml_dtypes.bfloat16)
    mones[:, bh_idx, bh_idx] = 1.0
    in_maps = []
    for c in range(NCORES):
        b0 = c * BLOC
        ks = key[b0 : b0 + BLOC, :, :lt, :].astype(ml_dtypes.bfloat16)
        # [bh, j, l', d] -> [j, d, bh, l']
        kj = ks.reshape(NBH, lt // LB, LB, D)
        kt = np.ascontiguousarray(kj.transpose(1, 3, 0, 2))
        qc = query[b0 : b0 + BLOC].reshape(NBH, D)  # [bh, d] f32
        mq = np.zeros((D, NBH, NBH), ml_dtypes.bfloat16)
        mq[:, bh_idx, bh_idx] = qc.T.astype(ml_dtypes.bfloat16)
        mf = np.repeat(
            mask[b0 : b0 + BLOC, :lt].astype(ml_dtypes.bfloat16), H, axis=0
        )  # [bh, l]
        in_maps.append(
            {
                "query": np.ascontiguousarray(query[b0 : b0 + BLOC], np.float32),
                "keyT": kt,
                "mq": mq,
                "mones": mones,
                "maskf": mf,
            }
        )
    return in_maps


def _run(query, key, mask, trace=False, nj=L // LB):
    nc = _get_module(nj)
    in_maps = stage_inputs(query, key, mask, nj)
    res = run_bass_kernel_spmd(
        nc, in_maps, core_ids=list(range(NCORES)), trace=trace
    )
    out = np.concatenate([r["out"] for r in res.results], axis=0).astype(np.float32)
    return out, res


def kernel(query, key, mask):
    out, _ = _run(np.asarray(query), np.asarray(key), np.asarray(mask))
    return out
